# revision 93
# baseline (speedup 1.0000x reference)
"""Multi-head attention (B=4, S=1024, H=1024, 16 heads) on 8 TRN2 NeuronCores.

Sharding: core c = (batch b = c//2, head-group g = c%2). Each core computes
attention for its batch over 8 of the 16 heads (a 512-wide column slice of
the QKV projections) plus the matching row-slice of the output projection.
The host sums the partial output projections (2 cores x 2 on-chip partials)
and adds bo.

On-core dataflow (inputs shipped bf16; QK logits in f32r; the attention-
weight matmul in bf16; psum accumulation fp32):
  QT[hd,s] = Wq_g^T x^T (+bq)   KT likewise (+bk)    V[t,hd] = x Wv_g (+bv)
  logitsT[t,s] per head: d=64 contraction, two heads packed in the PE via
                         tile_position row groups; heads processed serially
                         so each head's AV starts a half-pair earlier
  expT = exp(logitsT/8 + mask*NEG_INF)  (bf16; mask as per-partition bias)
  AVT[d,s] & colsum = [V_h | 1]^T @ expT (ones column makes psum row 64 the
                                          softmax denominator)
  attnT = AVT * (1/colsum)   (reciprocal on DVE, partition-broadcast on the
                              otherwise-idle Pool engine)
  out partials: head-pairs 0-1 ship mid-phase as a separate partial output
  (summed on the host) so only head-pairs 2-3 remain after the last AV --
  the tail is paced by the serial DMA bus, so B-chunks pair up per DMA.

Performance notes: all inputs packed into one DRAM tensor (HWDGE descriptor
generation is serial, ~0.63us per DMA; few big DMAs win), first proj blocks
interleave wq/xT per k-chunk so compute starts after 320KB, outputs in bf16.
"""
import sys

sys.path.insert(0, "/opt/trn_rl_repo")

import ml_dtypes
import numpy as np

import concourse.bass as bass
import concourse.mybir as mybir
import concourse.tile as tile
from concourse import bacc
from concourse.bass_utils import run_bass_kernel_spmd

F32 = mybir.dt.float32
F32R = mybir.dt.float32r
BF16 = mybir.dt.bfloat16
EXPTYPE = BF16

B, S, H = 4, 1024, 1024
NH, HD = 16, 64
HPG = 8            # heads per group (per core)
GW = HPG * HD      # 512: group width
NEG_INF = -2.0 ** 32
NCORES = 8
HC = H // 128      # 8 contraction chunks over hidden
TC = S // 128      # 8 chunks over key positions t
SB = S // 512      # 2 halves of the s (query) axis

Exp = mybir.ActivationFunctionType.Exp


def _build(nrep=1):
    nc = bacc.Bacc("TRN2", target_bir_lowering=False, debug=False)

    # all big inputs packed into one tensor (per-partition layout:
    # wq-b0 | xT-sh0 | wk-b0 | xT-sh1 | wq-b123 | wk-b123 | wv), so the
    # whole load is 3 big DMAs -- HWDGE descriptor generation (~0.63us per
    # DMA) is a serial resource.
    allin = nc.dram_tensor("allin", [128, 20480], BF16, kind="ExternalInput")
    wo = nc.dram_tensor("wo", [GW, S], BF16, kind="ExternalInput")
    # mask row | bq | bk (f32): one small DMA
    smalls = nc.dram_tensor("smalls", [128, TC + 8], F32, kind="ExternalInput")
    bv1 = nc.dram_tensor("bv1", [1, GW], F32, kind="ExternalInput")
    out = nc.dram_tensor("out", [S, H], BF16, kind="ExternalOutput")
    outA = nc.dram_tensor("outA", [S, H], BF16, kind="ExternalOutput")

    with tile.TileContext(nc, pool_alloc_mode="stack") as tc:
      for _rep in range(nrep):
          # Pool releases must be LIFO, so the three big input pools
          # (xT/wqk/wv) are created LAST: they release mid-kernel (stack
          # rewinds) and p_wo then reuses their space.
          misc_cm = tc.tile_pool(name="misc", bufs=1); misc = misc_cm.__enter__()
          qkt_cm = tc.tile_pool(name="p_qkt", bufs=1); p_qkt = qkt_cm.__enter__()
          v_cm = tc.tile_pool(name="p_v", bufs=1); p_v = v_cm.__enter__()
          exp_cm = tc.tile_pool(name="p_exp", bufs=3); p_exp = exp_cm.__enter__()
          attn_cm = tc.tile_pool(name="p_attn", bufs=1)
          p_attn = attn_cm.__enter__()
          nrm_cm = tc.tile_pool(name="p_nrm", bufs=2); p_nrm = nrm_cm.__enter__()
          o_cm = tc.tile_pool(name="p_o", bufs=3); p_o = o_cm.__enter__()
          xT_cm = tc.tile_pool(name="p_xT", bufs=1); p_xT = xT_cm.__enter__()
          wqk_cm = tc.tile_pool(name="p_wqk", bufs=1); p_wqk = wqk_cm.__enter__()
          wv_cm = tc.tile_pool(name="p_wv", bufs=1); p_wv = wv_cm.__enter__()
          late = {"p_nrm": p_nrm}
          lgps_cm = tc.tile_pool(name="ps_lg", bufs=2, space="PSUM")
          ps_lg = lgps_cm.__enter__()
          avps_cm = tc.tile_pool(name="ps_av", bufs=2, space="PSUM")
          ps_av = avps_cm.__enter__()
          qkvps_cm = tc.tile_pool(name="ps_qkv", bufs=2, space="PSUM")
          ps_qkv = qkvps_cm.__enter__()

          # ---- input loads: 3 big DMAs (SP) + 1 small (ACT). First
          # proj matmuls need only segment A (wq-b0 + xT-sh0).
          ain = p_xT.tile([128, 20480], BF16, tag="allin")
          nc.sync.dma_start(out=ain[:, 0:1280], in_=allin.ap()[:, 0:1280])
          nc.sync.dma_start(out=ain[:, 1280:3200], in_=allin.ap()[:, 1280:3200])
          nc.sync.dma_start(out=ain[:, 3200:5120], in_=allin.ap()[:, 3200:5120])
          nc.sync.dma_start(out=ain[:, 6144:8192], in_=allin.ap()[:, 6144:8192])
          nc.sync.dma_start(out=ain[:, 5120:6144], in_=allin.ap()[:, 5120:6144])
          nc.sync.dma_start(out=ain[:, 8192:10240], in_=allin.ap()[:, 8192:10240])
          nc.sync.dma_start(out=ain[:, 10240:14336], in_=allin.ap()[:, 10240:14336])
          nc.sync.dma_start(out=ain[:, 14336:20480], in_=allin.ap()[:, 14336:20480])
          sm = misc.tile([128, TC + 8], F32, tag="smalls")
          nc.scalar.dma_start(out=sm, in_=smalls.ap())
          mraw = sm[:, 0:TC]
          maskb = misc.tile([128, TC], F32, tag="maskb")
          nc.vector.tensor_scalar_mul(maskb, mraw, NEG_INF)
          bq_sb = sm[:, TC:TC + 4]
          bk_sb = sm[:, TC + 4:TC + 8]
          bv_bc = misc.tile([128, GW], F32, tag="bv")
          nc.scalar.dma_start(out=bv_bc, in_=bv1[0:1, :].to_broadcast((128, GW)))

          # segment A interleaves [wq0_c (128) | xT-sh0_c (512)] x 8 so the
          # first proj matmuls start after 320KB; then wk-b0 (small, for
          # K0-sh0), xT-sh1, wv (v_chunks are early PE filler), wq/wk b123.
          segA = ain[:, 0:5120].rearrange("p (c g) -> p c g", c=HC)

          def _wq(blk):
              if blk == 0:
                  return segA[:, :, 0:128]
              off = 14336 + (blk - 1) * 1024
              return ain[:, off:off + 1024].rearrange("p (c m) -> p c m", c=HC)

          def _wk(blk):
              if blk == 0:
                  return ain[:, 5120:6144].rearrange("p (c m) -> p c m", c=HC)
              off = 17408 + (blk - 1) * 1024
              return ain[:, off:off + 1024].rearrange("p (c m) -> p c m", c=HC)

          def _xT(sh):
              if sh == 0:
                  return segA[:, :, 128:640]
              return ain[:, 6144:10240].rearrange("p (c s) -> p c s", c=HC)

          wv_sb = ain[:, 10240:14336].rearrange("p (c m) -> p c m", c=HC)
          wo_sb = p_wv.tile([128, 4, S], BF16, tag="wo")
          nc.sync.dma_start(out=wo_sb, in_=wo.ap().rearrange("(c p) n -> p c n", p=128))
          QT_sb = p_qkt.tile([128, 4, S], F32R, tag="QT")
          KT_sb = p_qkt.tile([128, 4, S], F32R, tag="KT")
          V_sb = p_v.tile([128, TC, HPG, HD + 1], EXPTYPE, tag="V")
          nc.gpsimd.memset(V_sb[:, :, :, HD:HD + 1], 1.0)
          late["attnT"] = p_attn.tile([128, 4, S], BF16, tag="attnT", name="attnT")

          def proj_half(dst, blk, sh, wview, b_sb):
              """dst[:, blk, sh-half] (+bias) = block of Wg^T x^T."""
              ps = ps_qkv.tile([128, 512], F32, tag="mm512")
              w = wview(blk)
              xv = _xT(sh)
              for c in range(HC):
                  nc.tensor.matmul(
                      ps, w[:, c, :], xv[:, c, :],
                      start=(c == 0), stop=(c == HC - 1))
              nc.vector.tensor_scalar_add(
                  dst[:, blk, sh * 512:(sh + 1) * 512], ps, b_sb[:, blk:blk + 1])

          def v_chunk(tcn):
              """V_sb[:, tcn, :, 0:64] (+bv) = rows 128*tcn.. of x Wv_g."""
              ps = ps_qkv.tile([128, 512], F32, tag="mm512")
              xv = _xT(tcn // 4)
              ts = (tcn % 4) * 128
              for c in range(HC):
                  nc.tensor.matmul(
                      ps, xv[:, c, ts:ts + 128], wv_sb[:, c, :],
                      start=(c == 0), stop=(c == HC - 1))
              nc.vector.tensor_add(
                  V_sb[:, tcn, :, 0:HD],
                  ps.rearrange("p (h d) -> p h d", h=HPG),
                  bv_bc.rearrange("p (h d) -> p h d", h=HPG))

          def logits_head(h, tcn, expT_h):
              """d=64 logit matmuls (both s-halves) + exp for head h chunk
              tcn."""
              pair, off = h // 2, (h % 2) * 64
              lg = ps_lg.tile([128, 1024], F32, tag="lg")
              for sh in range(SB):
                  nc.tensor.matmul(
                      lg[:, sh * 512:(sh + 1) * 512],
                      KT_sb[off:off + 64, pair, tcn * 128:(tcn + 1) * 128],
                      QT_sb[off:off + 64, pair, sh * 512:(sh + 1) * 512],
                      start=True, stop=True, tile_position=(off, 0))
              nc.scalar.activation(
                  out=expT_h[:, tcn, :], in_=lg, func=Exp,
                  bias=maskb[:, tcn:tcn + 1], scale=0.125)

          def av_head_half(h, expT_h, sh):
              """attnT rows for head h, s-half sh = normalized V_h^T @ expT_h.
              The denominator reciprocal is broadcast across partitions on
              the otherwise-idle Pool engine."""
              off = (h % 2) * 64
              pav = ps_av.tile([HD + 1, 512], F32, tag="av",
                               name=f"pav{h}_{sh}")
              for tcn in range(TC):
                  nc.tensor.matmul(
                      pav, V_sb[:, tcn, h, :],
                      expT_h[:, tcn, sh * 512:(sh + 1) * 512],
                      start=(tcn == 0), stop=(tcn == TC - 1))
              recip = late["p_nrm"].tile([1, 512], F32, tag="recip")
              bcast = late["p_nrm"].tile([HD, 512], F32, tag="bcast")
              att = late["attnT"][off:off + HD, h // 2,
                                  sh * 512:(sh + 1) * 512]
              if h == HPG - 1:
                  # the last head gates the tail: halve the normalize chain
                  # so out-proj st chunks start once their s-columns land
                  for q in range(2):
                      cs = slice(q * 256, (q + 1) * 256)
                      nc.vector.reciprocal(recip[:, cs], pav[HD:HD + 1, cs])
                      nc.gpsimd.partition_broadcast(
                          bcast[:, cs], recip[:, cs], channels=HD)
                      nc.vector.tensor_mul(att[:, cs], pav[0:HD, cs],
                                           bcast[:, cs])
              else:
                  nc.vector.reciprocal(recip, pav[HD:HD + 1, :])
                  nc.gpsimd.partition_broadcast(bcast, recip, channels=HD)
                  nc.vector.tensor_mul(att, pav[0:HD, :], bcast)

          def out_projA(st, nh, blks, dst):
              """partial output (head-pairs 0-1) for chunk (st, nh): runs
              mid-phase through the idle mm512 ring; the host sums the two
              partial outputs, so only head-pairs 2-3 remain for the tail."""
              po = ps_qkv.tile([128, 512], F32, tag="mm512",
                               name=f"poA{st}_{nh}_{blks[0]}")
              for i, blk in enumerate(blks):
                  nc.tensor.matmul(
                      po, late["attnT"][:, blk, st * 128:(st + 1) * 128],
                      wo_sb[:, blk, nh * 512:(nh + 1) * 512],
                      start=(i == 0), stop=(i == len(blks) - 1))
              if nh == 0:
                  late[f"oa{st}"] = p_o.tile([128, 1024], BF16, tag="oa",
                                             name=f"oa{st}")
              oa = late[f"oa{st}"]
              cs = slice(nh * 512, (nh + 1) * 512)
              if (2 * st + nh) % 2 == 0:
                  nc.vector.tensor_copy(oa[:, cs], po)
              else:
                  nc.scalar.copy(oa[:, cs], po)
              if nh == 1:
                  nc.sync.dma_start(out=dst[st * 128:(st + 1) * 128, :], in_=oa)

          def out_proj2(st2):
              """B partial (head-pairs 2-3) for st chunks 2*st2, 2*st2+1:
              two chunks share one SBUF tile and one DMA -- HWDGE and the
              DMA bus are serial and pace the tail."""
              last = st2 == TC // 2 - 1
              o2 = p_o.tile([128, 2, 1024], BF16, tag="o", name=f"o2_{st2}")
              for sl in range(2):
                  st = 2 * st2 + sl
                  pool = late["ps_po"] if st % 3 == 2 else ps_lg
                  po = pool.tile([128, 1024], F32, tag="lg", name=f"po{st}")
                  for nh in range(SB):
                      for blk in (2, 3):
                          nc.tensor.matmul(
                              po[:, nh * 512:(nh + 1) * 512],
                              late["attnT"][:, blk, st * 128:(st + 1) * 128],
                              wo_sb[:, blk, nh * 512:(nh + 1) * 512],
                              start=(blk == 2), stop=(blk == 3))
                  if last:
                      nc.vector.tensor_copy(o2[:, sl, 0:512], po[:, 0:512])
                      nc.scalar.copy(o2[:, sl, 512:1024], po[:, 512:1024])
                  elif sl == 0:
                      nc.vector.tensor_copy(o2[:, 0, :], po)
                  else:
                      nc.scalar.copy(o2[:, 1, :], po)
              if last:
                  for sl in range(2):
                      st = 2 * st2 + sl
                      nc.sync.dma_start(
                          out=out[st * 128:(st + 1) * 128, :], in_=o2[:, sl, :])
              else:
                  nc.sync.dma_start(
                      out=out.ap()[st2 * 256:(st2 + 1) * 256, :]
                            .rearrange("(two p) n -> p two n", two=2),
                      in_=o2)


          # ---------------- emission ----------------
          for sh in range(SB):
              proj_half(QT_sb, 0, sh, _wq, bq_sb)
          for sh in range(SB):
              proj_half(KT_sb, 0, sh, _wk, bk_sb)

          expT = {}
          for h in range(HPG):
              expT[h] = p_exp.tile([128, TC, S], EXPTYPE, tag="expT",
                                   name=f"expT{h}")
              for tcn in range(TC):
                  # interleaved fill work (emission order = scheduler
                  # priority; real ordering comes from the dataflow)
                  if h == 0:
                      if tcn in (0, 2, 4, 6):
                          v_chunk(tcn // 2)
                      elif tcn == 1:
                          proj_half(QT_sb, 1, 0, _wq, bq_sb)
                      elif tcn == 3:
                          proj_half(QT_sb, 1, 1, _wq, bq_sb)
                      elif tcn == 5:
                          proj_half(KT_sb, 1, 0, _wk, bk_sb)
                      elif tcn == 7:
                          proj_half(KT_sb, 1, 1, _wk, bk_sb)
                  elif h == 1 and tcn < 4:
                      v_chunk(4 + tcn)
                  elif h in (2, 4) and tcn in (1, 3, 5, 7):
                      blk = h // 2 + 1
                      wv_, b_sb_ = (_wq, bq_sb) if tcn < 4 else (_wk, bk_sb)
                      dst = QT_sb if tcn < 4 else KT_sb
                      proj_half(dst, blk, (tcn % 4) // 2, wv_, b_sb_)
                  if h == 1:
                      if tcn == 4:
                          av_head_half(0, expT[0], 0)
                      elif tcn == 6:
                          av_head_half(0, expT[0], 1)
                  elif h >= 2:
                      if tcn == 2:
                          av_head_half(h - 1, expT[h - 1], 0)
                      elif tcn == 5:
                          av_head_half(h - 1, expT[h - 1], 1)
                  if h >= 5 and tcn % 2 == 0:
                      u = (h - 5) * 4 + tcn // 2
                      if u < 16:
                          out_projA(u // 2, u % 2, (0, 1), outA)
                  logits_head(h, tcn, expT[h])

          wv_cm.__exit__(None, None, None)
          wqk_cm.__exit__(None, None, None)
          xT_cm.__exit__(None, None, None)

          attnT = late["attnT"]

          for u in range(12, 16):
              out_projA(u // 2, u % 2, (0, 1), outA)
          qkvps_cm.__exit__(None, None, None)
          pops_cm = tc.tile_pool(name="ps_po", bufs=1, space="PSUM")
          late["ps_po"] = pops_cm.__enter__()
          # last head's AVs; the sh1 normalize chain overlaps the st 0-3
          # out-proj matmuls (which only need sh0 rows)
          av_head_half(7, expT[7], 0)
          av_head_half(7, expT[7], 1)
          for st2 in range(TC // 2):
              out_proj2(st2)

          pops_cm.__exit__(None, None, None)
          for cm in (o_cm, nrm_cm, attn_cm, exp_cm, v_cm, qkt_cm,
                     misc_cm, avps_cm, lgps_cm):
              cm.__exit__(None, None, None)

    nc.compile()
    return nc


_NC = {}


def _get_nc(nrep=1):
    if nrep not in _NC:
        _NC[nrep] = _build(nrep)
    return _NC[nrep]


def kernel(x, mask, Wq, bq, Wk, bk, Wv, bv, Wo, bo, _trace=False):
    x = np.asarray(x, dtype=np.float32)
    mask = np.asarray(mask, dtype=np.float32)
    Wq, Wk, Wv, Wo = (np.asarray(w, dtype=np.float32) for w in (Wq, Wk, Wv, Wo))
    bq, bk, bv, bo = (np.asarray(b_, dtype=np.float32) for b_ in (bq, bk, bv, bo))

    nc = _get_nc()

    def _blkmaj(w):
        # [H, GW-slice] -> per-partition [blk 4, c 8, 128]: value (p, blk, c, m)
        # = w[c*128 + p, blk*128 + m]
        r = w.reshape(HC, 128, 4, 128)            # [c, p, blk, m]
        return r.transpose(1, 2, 0, 3)            # [p, blk, c, m]

    def _shmaj(xTb):
        # xT [H, S] -> per-partition [sh 2, c 8, 512]
        r = xTb.reshape(HC, 128, 2, 512)          # [c, p, sh, s]
        return r.transpose(1, 2, 0, 3)            # [p, sh, c, s]

    in_maps = []
    for c in range(NCORES):
        b, g = c // 2, c % 2
        sl = slice(g * GW, (g + 1) * GW)
        bf = ml_dtypes.bfloat16
        wqb = _blkmaj(Wq[:, sl].astype(bf))       # [p, 4, 8, 128]
        wkb = _blkmaj(Wk[:, sl].astype(bf))
        xsh = _shmaj(np.ascontiguousarray(x[b].T).astype(bf))  # [p, 2, 8, 512]
        wvb = Wv[:, sl].astype(bf).reshape(HC, 128, GW).transpose(1, 0, 2)
        segA = np.concatenate([wqb[:, 0], xsh[:, 0]], axis=2)  # [p, c, 640]
        allin = np.concatenate([
            segA.reshape(128, -1),                # wq-b0|xT-sh0 interleaved
            wkb[:, 0].reshape(128, -1),           # wk blk0   1024
            xsh[:, 1].reshape(128, -1),           # xT sh1    4096
            wvb.reshape(128, -1),                 # wv        4096
            wqb[:, 1:4].reshape(128, -1),         # wq blk123 3072
            wkb[:, 1:4].reshape(128, -1),         # wk blk123 3072
        ], axis=1)
        smalls = np.concatenate([
            mask[b, 0, 0, :].reshape(TC, 128).T,
            bq[sl].reshape(4, 128).T,
            bk[sl].reshape(4, 128).T,
        ], axis=1).astype(np.float32)
        in_maps.append({
            "allin": np.ascontiguousarray(allin),
            "wo": np.ascontiguousarray(Wo[sl, :]).astype(bf),
            "smalls": np.ascontiguousarray(smalls),
            "bv1": np.ascontiguousarray(bv[sl]).reshape(1, GW).astype(np.float32),
        })
    # First execution after NEFF load can race engine table initialization
    # (observed: garbage exp output on run 1 only). Warm up, then run.
    run_bass_kernel_spmd(nc, in_maps, core_ids=list(range(NCORES)))
    res = run_bass_kernel_spmd(
        nc, in_maps, core_ids=list(range(NCORES)), trace=_trace)
    kernel.last_results = res
    parts = [np.asarray(res.results[c]["out"]).astype(np.float32)
             + np.asarray(res.results[c]["outA"]).astype(np.float32)
             for c in range(NCORES)]
    return np.stack(
        [parts[2 * b] + parts[2 * b + 1] + bo for b in range(B)]
    ).astype(np.float32)



# revision 99
# speedup vs baseline: 1.0012x; 1.0012x over previous
"""Multi-head attention (B=4, S=1024, H=1024, 16 heads) on 8 TRN2 NeuronCores.

Sharding: core c = (batch b = c//2, head-group g = c%2). Each core computes
attention for its batch over 8 of the 16 heads (a 512-wide column slice of
the QKV projections) plus the matching row-slice of the output projection.
The host sums the partial output projections (2 cores x 2 on-chip partials)
and adds bo.

On-core dataflow (inputs shipped bf16; QK logits in f32r; the attention-
weight matmul in bf16; psum accumulation fp32):
  QT[hd,s] = Wq_g^T x^T (+bq)   KT likewise (+bk)    V[t,hd] = x Wv_g (+bv)
  logitsT[t,s] per head: d=64 contraction, two heads packed in the PE via
                         tile_position row groups; heads processed serially
                         so each head's AV starts a half-pair earlier
  expT = exp(logitsT/8 + mask*NEG_INF)  (bf16; mask as per-partition bias)
  AVT[d,s] & colsum = [V_h | 1]^T @ expT (ones column makes psum row 64 the
                                          softmax denominator)
  attnT = AVT * (1/colsum)   (reciprocal on DVE, partition-broadcast on the
                              otherwise-idle Pool engine)
  out partials: head-pairs 0-1 ship mid-phase as a separate partial output
  (summed on the host) so only head-pairs 2-3 remain after the last AV --
  the tail is paced by the serial DMA bus, so B-chunks pair up per DMA.

Performance notes: all inputs packed into one DRAM tensor (HWDGE descriptor
generation is serial, ~0.63us per DMA; few big DMAs win), first proj blocks
interleave wq/xT per k-chunk so compute starts after 320KB, outputs in bf16.
"""
import sys

sys.path.insert(0, "/opt/trn_rl_repo")

import ml_dtypes
import numpy as np

import concourse.bass as bass
import concourse.mybir as mybir
import concourse.tile as tile
from concourse import bacc
from concourse.bass_utils import run_bass_kernel_spmd

F32 = mybir.dt.float32
F32R = mybir.dt.float32r
BF16 = mybir.dt.bfloat16
EXPTYPE = BF16

B, S, H = 4, 1024, 1024
NH, HD = 16, 64
HPG = 8            # heads per group (per core)
GW = HPG * HD      # 512: group width
NEG_INF = -2.0 ** 32
NCORES = 8
HC = H // 128      # 8 contraction chunks over hidden
TC = S // 128      # 8 chunks over key positions t
SB = S // 512      # 2 halves of the s (query) axis

Exp = mybir.ActivationFunctionType.Exp


def _build(nrep=1):
    nc = bacc.Bacc("TRN2", target_bir_lowering=False, debug=False)

    # all big inputs packed into one tensor (per-partition layout:
    # wq-b0 | xT-sh0 | wk-b0 | xT-sh1 | wq-b123 | wk-b123 | wv), so the
    # whole load is 3 big DMAs -- HWDGE descriptor generation (~0.63us per
    # DMA) is a serial resource.
    allin = nc.dram_tensor("allin", [128, 20480], BF16, kind="ExternalInput")
    wo = nc.dram_tensor("wo", [GW, S], BF16, kind="ExternalInput")
    # mask row | bq | bk (f32): one small DMA
    smalls = nc.dram_tensor("smalls", [128, TC + 8], F32, kind="ExternalInput")
    bv1 = nc.dram_tensor("bv1", [1, GW], F32, kind="ExternalInput")
    out = nc.dram_tensor("out", [S, H], BF16, kind="ExternalOutput")
    outA = nc.dram_tensor("outA", [S, H], BF16, kind="ExternalOutput")

    with tile.TileContext(nc, pool_alloc_mode="stack") as tc:
      for _rep in range(nrep):
          # Pool releases must be LIFO, so the three big input pools
          # (xT/wqk/wv) are created LAST: they release mid-kernel (stack
          # rewinds) and p_wo then reuses their space.
          misc_cm = tc.tile_pool(name="misc", bufs=1); misc = misc_cm.__enter__()
          qkt_cm = tc.tile_pool(name="p_qkt", bufs=1); p_qkt = qkt_cm.__enter__()
          v_cm = tc.tile_pool(name="p_v", bufs=1); p_v = v_cm.__enter__()
          exp_cm = tc.tile_pool(name="p_exp", bufs=3); p_exp = exp_cm.__enter__()
          attn_cm = tc.tile_pool(name="p_attn", bufs=1)
          p_attn = attn_cm.__enter__()
          nrm_cm = tc.tile_pool(name="p_nrm", bufs=2); p_nrm = nrm_cm.__enter__()
          o_cm = tc.tile_pool(name="p_o", bufs=3); p_o = o_cm.__enter__()
          xT_cm = tc.tile_pool(name="p_xT", bufs=1); p_xT = xT_cm.__enter__()
          wqk_cm = tc.tile_pool(name="p_wqk", bufs=1); p_wqk = wqk_cm.__enter__()
          wv_cm = tc.tile_pool(name="p_wv", bufs=1); p_wv = wv_cm.__enter__()
          late = {"p_nrm": p_nrm}
          lgps_cm = tc.tile_pool(name="ps_lg", bufs=2, space="PSUM")
          ps_lg = lgps_cm.__enter__()
          avps_cm = tc.tile_pool(name="ps_av", bufs=2, space="PSUM")
          ps_av = avps_cm.__enter__()
          qkvps_cm = tc.tile_pool(name="ps_qkv", bufs=2, space="PSUM")
          ps_qkv = qkvps_cm.__enter__()

          # ---- input loads: 3 big DMAs (SP) + 1 small (ACT). First
          # proj matmuls need only segment A (wq-b0 + xT-sh0).
          ain = p_xT.tile([128, 20480], BF16, tag="allin")
          nc.sync.dma_start(out=ain[:, 0:1280], in_=allin.ap()[:, 0:1280])
          nc.sync.dma_start(out=ain[:, 1280:3200], in_=allin.ap()[:, 1280:3200])
          nc.sync.dma_start(out=ain[:, 3200:5120], in_=allin.ap()[:, 3200:5120])
          nc.sync.dma_start(out=ain[:, 6144:8192], in_=allin.ap()[:, 6144:8192])
          nc.sync.dma_start(out=ain[:, 5120:6144], in_=allin.ap()[:, 5120:6144])
          nc.sync.dma_start(out=ain[:, 8192:10240], in_=allin.ap()[:, 8192:10240])
          nc.sync.dma_start(out=ain[:, 10240:14336], in_=allin.ap()[:, 10240:14336])
          nc.sync.dma_start(out=ain[:, 14336:20480], in_=allin.ap()[:, 14336:20480])
          sm = misc.tile([128, TC + 8], F32, tag="smalls")
          nc.scalar.dma_start(out=sm, in_=smalls.ap())
          mraw = sm[:, 0:TC]
          maskb = misc.tile([128, TC], F32, tag="maskb")
          nc.vector.tensor_scalar_mul(maskb, mraw, NEG_INF)
          bq_sb = sm[:, TC:TC + 4]
          bk_sb = sm[:, TC + 4:TC + 8]
          bv_bc = misc.tile([128, GW], F32, tag="bv")
          nc.scalar.dma_start(out=bv_bc, in_=bv1[0:1, :].to_broadcast((128, GW)))

          # segment A interleaves [wq0_c (128) | xT-sh0_c (512)] x 8 so the
          # first proj matmuls start after 320KB; then wk-b0 (small, for
          # K0-sh0), xT-sh1, wv (v_chunks are early PE filler), wq/wk b123.
          segA = ain[:, 0:5120].rearrange("p (c g) -> p c g", c=HC)

          def _wq(blk):
              if blk == 0:
                  return segA[:, :, 0:128]
              off = 14336 + (blk - 1) * 1024
              return ain[:, off:off + 1024].rearrange("p (c m) -> p c m", c=HC)

          def _wk(blk):
              if blk == 0:
                  return ain[:, 5120:6144].rearrange("p (c m) -> p c m", c=HC)
              off = 17408 + (blk - 1) * 1024
              return ain[:, off:off + 1024].rearrange("p (c m) -> p c m", c=HC)

          def _xT(sh):
              if sh == 0:
                  return segA[:, :, 128:640]
              return ain[:, 6144:10240].rearrange("p (c s) -> p c s", c=HC)

          wv_sb = ain[:, 10240:14336].rearrange("p (c m) -> p c m", c=HC)
          wo_sb = p_wv.tile([128, 4, S], BF16, tag="wo")
          nc.sync.dma_start(out=wo_sb, in_=wo.ap().rearrange("(c p) n -> p c n", p=128))
          QT_sb = p_qkt.tile([128, 4, S], F32R, tag="QT")
          KT_sb = p_qkt.tile([128, 4, S], F32R, tag="KT")
          V_sb = p_v.tile([128, TC, HPG, HD + 1], EXPTYPE, tag="V")
          nc.gpsimd.memset(V_sb[:, :, :, HD:HD + 1], 1.0)
          late["attnT"] = p_attn.tile([128, 4, S], BF16, tag="attnT", name="attnT")

          def proj_half(dst, blk, sh, wview, b_sb):
              """dst[:, blk, sh-half] (+bias) = block of Wg^T x^T."""
              ps = ps_qkv.tile([128, 512], F32, tag="mm512")
              w = wview(blk)
              xv = _xT(sh)
              for c in range(HC):
                  nc.tensor.matmul(
                      ps, w[:, c, :], xv[:, c, :],
                      start=(c == 0), stop=(c == HC - 1))
              nc.vector.tensor_scalar_add(
                  dst[:, blk, sh * 512:(sh + 1) * 512], ps, b_sb[:, blk:blk + 1])

          def v_chunk(tcn):
              """V_sb[:, tcn, :, 0:64] (+bv) = rows 128*tcn.. of x Wv_g."""
              ps = ps_qkv.tile([128, 512], F32, tag="mm512")
              xv = _xT(tcn // 4)
              ts = (tcn % 4) * 128
              for c in range(HC):
                  nc.tensor.matmul(
                      ps, xv[:, c, ts:ts + 128], wv_sb[:, c, :],
                      start=(c == 0), stop=(c == HC - 1))
              nc.vector.tensor_add(
                  V_sb[:, tcn, :, 0:HD],
                  ps.rearrange("p (h d) -> p h d", h=HPG),
                  bv_bc.rearrange("p (h d) -> p h d", h=HPG))

          def logits_head(h, tcn, expT_h):
              """d=64 logit matmuls (both s-halves) + exp for head h chunk
              tcn."""
              pair, off = h // 2, (h % 2) * 64
              lg = ps_lg.tile([128, 1024], F32, tag="lg")
              for sh in range(SB):
                  nc.tensor.matmul(
                      lg[:, sh * 512:(sh + 1) * 512],
                      KT_sb[off:off + 64, pair, tcn * 128:(tcn + 1) * 128],
                      QT_sb[off:off + 64, pair, sh * 512:(sh + 1) * 512],
                      start=True, stop=True, tile_position=(off, 0))
              nc.scalar.activation(
                  out=expT_h[:, tcn, :], in_=lg, func=Exp,
                  bias=maskb[:, tcn:tcn + 1], scale=0.125)

          def av_head_half(h, expT_h, sh):
              """attnT rows for head h, s-half sh = normalized V_h^T @ expT_h.
              The denominator reciprocal is broadcast across partitions on
              the otherwise-idle Pool engine."""
              off = (h % 2) * 64
              pav = ps_av.tile([HD + 1, 512], F32, tag="av",
                               name=f"pav{h}_{sh}")
              for tcn in range(TC):
                  nc.tensor.matmul(
                      pav, V_sb[:, tcn, h, :],
                      expT_h[:, tcn, sh * 512:(sh + 1) * 512],
                      start=(tcn == 0), stop=(tcn == TC - 1))
              recip = late["p_nrm"].tile([1, 512], F32, tag="recip")
              bcast = late["p_nrm"].tile([HD, 512], F32, tag="bcast")
              att = late["attnT"][off:off + HD, h // 2,
                                  sh * 512:(sh + 1) * 512]
              if h == HPG - 1:
                  # the last head gates the tail: halve the normalize chain
                  # so out-proj st chunks start once their s-columns land
                  for q in range(2):
                      cs = slice(q * 256, (q + 1) * 256)
                      nc.vector.reciprocal(recip[:, cs], pav[HD:HD + 1, cs])
                      nc.gpsimd.partition_broadcast(
                          bcast[:, cs], recip[:, cs], channels=HD)
                      nc.vector.tensor_mul(att[:, cs], pav[0:HD, cs],
                                           bcast[:, cs])
              else:
                  nc.vector.reciprocal(recip, pav[HD:HD + 1, :])
                  nc.gpsimd.partition_broadcast(bcast, recip, channels=HD)
                  nc.vector.tensor_mul(att, pav[0:HD, :], bcast)

          def out_projA(st, nh, blks, dst):
              """partial output (head-pairs 0-1) for chunk (st, nh): runs
              mid-phase through the idle mm512 ring; the host sums the two
              partial outputs, so only head-pairs 2-3 remain for the tail."""
              po = ps_qkv.tile([128, 512], F32, tag="mm512",
                               name=f"poA{st}_{nh}_{blks[0]}")
              for i, blk in enumerate(blks):
                  nc.tensor.matmul(
                      po, late["attnT"][:, blk, st * 128:(st + 1) * 128],
                      wo_sb[:, blk, nh * 512:(nh + 1) * 512],
                      start=(i == 0), stop=(i == len(blks) - 1))
              if nh == 0:
                  late[f"oa{st}"] = p_o.tile([128, 1024], BF16, tag="oa",
                                             name=f"oa{st}")
              oa = late[f"oa{st}"]
              cs = slice(nh * 512, (nh + 1) * 512)
              if (2 * st + nh) % 2 == 0:
                  nc.vector.tensor_copy(oa[:, cs], po)
              else:
                  nc.scalar.copy(oa[:, cs], po)
              if nh == 1:
                  nc.sync.dma_start(out=dst[st * 128:(st + 1) * 128, :], in_=oa)

          def out_proj2(st2):
              """B partial (head-pairs 2-3) for st chunks 2*st2, 2*st2+1:
              two chunks share one SBUF tile and one DMA -- HWDGE and the
              DMA bus are serial and pace the tail."""
              last = st2 == TC // 2 - 1
              o2 = p_o.tile([128, 2, 1024], BF16, tag="o", name=f"o2_{st2}")
              for sl in range(2):
                  st = 2 * st2 + sl
                  pool = late["ps_po"] if st % 3 == 0 else ps_lg
                  po = pool.tile([128, 1024], F32, tag="lg", name=f"po{st}")
                  for nh in range(SB):
                      for blk in (2, 3):
                          nc.tensor.matmul(
                              po[:, nh * 512:(nh + 1) * 512],
                              late["attnT"][:, blk, st * 128:(st + 1) * 128],
                              wo_sb[:, blk, nh * 512:(nh + 1) * 512],
                              start=(blk == 2), stop=(blk == 3))
                  if last:
                      nc.vector.tensor_copy(o2[:, sl, 0:512], po[:, 0:512])
                      nc.scalar.copy(o2[:, sl, 512:1024], po[:, 512:1024])
                  elif sl == 0:
                      nc.vector.tensor_copy(o2[:, 0, :], po)
                  else:
                      nc.scalar.copy(o2[:, 1, :], po)
              if last:
                  for sl in range(2):
                      st = 2 * st2 + sl
                      nc.sync.dma_start(
                          out=out[st * 128:(st + 1) * 128, :], in_=o2[:, sl, :])
              else:
                  nc.sync.dma_start(
                      out=out.ap()[st2 * 256:(st2 + 1) * 256, :]
                            .rearrange("(two p) n -> p two n", two=2),
                      in_=o2)


          # ---------------- emission ----------------
          for sh in range(SB):
              proj_half(QT_sb, 0, sh, _wq, bq_sb)
          for sh in range(SB):
              proj_half(KT_sb, 0, sh, _wk, bk_sb)

          expT = {}
          for h in range(HPG):
              expT[h] = p_exp.tile([128, TC, S], EXPTYPE, tag="expT",
                                   name=f"expT{h}")
              for tcn in range(TC):
                  # interleaved fill work (emission order = scheduler
                  # priority; real ordering comes from the dataflow)
                  if h == 0:
                      if tcn in (0, 2, 4, 6):
                          v_chunk(tcn // 2)
                      elif tcn == 1:
                          proj_half(QT_sb, 1, 0, _wq, bq_sb)
                      elif tcn == 3:
                          proj_half(QT_sb, 1, 1, _wq, bq_sb)
                      elif tcn == 5:
                          proj_half(KT_sb, 1, 0, _wk, bk_sb)
                      elif tcn == 7:
                          proj_half(KT_sb, 1, 1, _wk, bk_sb)
                  elif h == 1 and tcn < 4:
                      v_chunk(4 + tcn)
                  elif h in (2, 4) and tcn in (1, 3, 5, 7):
                      blk = h // 2 + 1
                      wv_, b_sb_ = (_wq, bq_sb) if tcn < 4 else (_wk, bk_sb)
                      dst = QT_sb if tcn < 4 else KT_sb
                      proj_half(dst, blk, (tcn % 4) // 2, wv_, b_sb_)
                  if h == 1:
                      if tcn == 4:
                          av_head_half(0, expT[0], 0)
                      elif tcn == 6:
                          av_head_half(0, expT[0], 1)
                  elif h >= 2:
                      if tcn == 2:
                          av_head_half(h - 1, expT[h - 1], 0)
                      elif tcn == 5:
                          av_head_half(h - 1, expT[h - 1], 1)
                  if h >= 5 and tcn % 2 == 0:
                      u = (h - 5) * 4 + tcn // 2
                      if u < 16:
                          out_projA(u // 2, u % 2, (0, 1), outA)
                  logits_head(h, tcn, expT[h])

          wv_cm.__exit__(None, None, None)
          wqk_cm.__exit__(None, None, None)
          xT_cm.__exit__(None, None, None)

          attnT = late["attnT"]

          for u in range(12, 16):
              out_projA(u // 2, u % 2, (0, 1), outA)
          qkvps_cm.__exit__(None, None, None)
          pops_cm = tc.tile_pool(name="ps_po", bufs=1, space="PSUM")
          late["ps_po"] = pops_cm.__enter__()
          # last head's AVs; the sh1 normalize chain overlaps the st 0-3
          # out-proj matmuls (which only need sh0 rows)
          av_head_half(7, expT[7], 0)
          av_head_half(7, expT[7], 1)
          for st2 in range(TC // 2):
              out_proj2(st2)

          pops_cm.__exit__(None, None, None)
          for cm in (o_cm, nrm_cm, attn_cm, exp_cm, v_cm, qkt_cm,
                     misc_cm, avps_cm, lgps_cm):
              cm.__exit__(None, None, None)

    nc.compile()
    return nc


_NC = {}


def _get_nc(nrep=1):
    if nrep not in _NC:
        _NC[nrep] = _build(nrep)
    return _NC[nrep]


def kernel(x, mask, Wq, bq, Wk, bk, Wv, bv, Wo, bo, _trace=False):
    x = np.asarray(x, dtype=np.float32)
    mask = np.asarray(mask, dtype=np.float32)
    Wq, Wk, Wv, Wo = (np.asarray(w, dtype=np.float32) for w in (Wq, Wk, Wv, Wo))
    bq, bk, bv, bo = (np.asarray(b_, dtype=np.float32) for b_ in (bq, bk, bv, bo))

    nc = _get_nc()

    def _blkmaj(w):
        # [H, GW-slice] -> per-partition [blk 4, c 8, 128]: value (p, blk, c, m)
        # = w[c*128 + p, blk*128 + m]
        r = w.reshape(HC, 128, 4, 128)            # [c, p, blk, m]
        return r.transpose(1, 2, 0, 3)            # [p, blk, c, m]

    def _shmaj(xTb):
        # xT [H, S] -> per-partition [sh 2, c 8, 512]
        r = xTb.reshape(HC, 128, 2, 512)          # [c, p, sh, s]
        return r.transpose(1, 2, 0, 3)            # [p, sh, c, s]

    in_maps = []
    for c in range(NCORES):
        b, g = c // 2, c % 2
        sl = slice(g * GW, (g + 1) * GW)
        bf = ml_dtypes.bfloat16
        wqb = _blkmaj(Wq[:, sl].astype(bf))       # [p, 4, 8, 128]
        wkb = _blkmaj(Wk[:, sl].astype(bf))
        xsh = _shmaj(np.ascontiguousarray(x[b].T).astype(bf))  # [p, 2, 8, 512]
        wvb = Wv[:, sl].astype(bf).reshape(HC, 128, GW).transpose(1, 0, 2)
        segA = np.concatenate([wqb[:, 0], xsh[:, 0]], axis=2)  # [p, c, 640]
        allin = np.concatenate([
            segA.reshape(128, -1),                # wq-b0|xT-sh0 interleaved
            wkb[:, 0].reshape(128, -1),           # wk blk0   1024
            xsh[:, 1].reshape(128, -1),           # xT sh1    4096
            wvb.reshape(128, -1),                 # wv        4096
            wqb[:, 1:4].reshape(128, -1),         # wq blk123 3072
            wkb[:, 1:4].reshape(128, -1),         # wk blk123 3072
        ], axis=1)
        smalls = np.concatenate([
            mask[b, 0, 0, :].reshape(TC, 128).T,
            bq[sl].reshape(4, 128).T,
            bk[sl].reshape(4, 128).T,
        ], axis=1).astype(np.float32)
        in_maps.append({
            "allin": np.ascontiguousarray(allin),
            "wo": np.ascontiguousarray(Wo[sl, :]).astype(bf),
            "smalls": np.ascontiguousarray(smalls),
            "bv1": np.ascontiguousarray(bv[sl]).reshape(1, GW).astype(np.float32),
        })
    # First execution after NEFF load can race engine table initialization
    # (observed: garbage exp output on run 1 only). Warm up, then run.
    run_bass_kernel_spmd(nc, in_maps, core_ids=list(range(NCORES)))
    res = run_bass_kernel_spmd(
        nc, in_maps, core_ids=list(range(NCORES)), trace=_trace)
    kernel.last_results = res
    parts = [np.asarray(res.results[c]["out"]).astype(np.float32)
             + np.asarray(res.results[c]["outA"]).astype(np.float32)
             for c in range(NCORES)]
    return np.stack(
        [parts[2 * b] + parts[2 * b + 1] + bo for b in range(B)]
    ).astype(np.float32)



# revision 100
# speedup vs baseline: 1.0049x; 1.0036x over previous
"""Multi-head attention (B=4, S=1024, H=1024, 16 heads) on 8 TRN2 NeuronCores.

Sharding: core c = (batch b = c//2, head-group g = c%2). Each core computes
attention for its batch over 8 of the 16 heads (a 512-wide column slice of
the QKV projections) plus the matching row-slice of the output projection.
The host sums the partial output projections (2 cores x 2 on-chip partials)
and adds bo.

On-core dataflow (inputs shipped bf16; QK logits in f32r; the attention-
weight matmul in bf16; psum accumulation fp32):
  QT[hd,s] = Wq_g^T x^T (+bq)   KT likewise (+bk)    V[t,hd] = x Wv_g (+bv)
  logitsT[t,s] per head: d=64 contraction, two heads packed in the PE via
                         tile_position row groups; heads processed serially
                         so each head's AV starts a half-pair earlier
  expT = exp(logitsT/8 + mask*NEG_INF)  (bf16; mask as per-partition bias)
  AVT[d,s] & colsum = [V_h | 1]^T @ expT (ones column makes psum row 64 the
                                          softmax denominator)
  attnT = AVT * (1/colsum)   (reciprocal on DVE, partition-broadcast on the
                              otherwise-idle Pool engine)
  out partials: head-pairs 0-1 ship mid-phase as a separate partial output
  (summed on the host) so only head-pairs 2-3 remain after the last AV --
  the tail is paced by the serial DMA bus, so B-chunks pair up per DMA.

Performance notes: all inputs packed into one DRAM tensor (HWDGE descriptor
generation is serial, ~0.63us per DMA; few big DMAs win), first proj blocks
interleave wq/xT per k-chunk so compute starts after 320KB, outputs in bf16.
"""
import sys

sys.path.insert(0, "/opt/trn_rl_repo")

import ml_dtypes
import numpy as np

import concourse.bass as bass
import concourse.mybir as mybir
import concourse.tile as tile
from concourse import bacc
from concourse.bass_utils import run_bass_kernel_spmd

F32 = mybir.dt.float32
F32R = mybir.dt.float32r
BF16 = mybir.dt.bfloat16
EXPTYPE = BF16

B, S, H = 4, 1024, 1024
NH, HD = 16, 64
HPG = 8            # heads per group (per core)
GW = HPG * HD      # 512: group width
NEG_INF = -2.0 ** 32
NCORES = 8
HC = H // 128      # 8 contraction chunks over hidden
TC = S // 128      # 8 chunks over key positions t
SB = S // 512      # 2 halves of the s (query) axis

Exp = mybir.ActivationFunctionType.Exp


def _build(nrep=1):
    nc = bacc.Bacc("TRN2", target_bir_lowering=False, debug=False)

    # all big inputs packed into one tensor (per-partition layout:
    # wq-b0 | xT-sh0 | wk-b0 | xT-sh1 | wq-b123 | wk-b123 | wv), so the
    # whole load is 3 big DMAs -- HWDGE descriptor generation (~0.63us per
    # DMA) is a serial resource.
    allin = nc.dram_tensor("allin", [128, 20480], BF16, kind="ExternalInput")
    wo = nc.dram_tensor("wo", [GW, S], BF16, kind="ExternalInput")
    # mask row | bq | bk (f32): one small DMA
    smalls = nc.dram_tensor("smalls", [128, TC + 8], F32, kind="ExternalInput")
    bv1 = nc.dram_tensor("bv1", [1, GW], F32, kind="ExternalInput")
    out = nc.dram_tensor("out", [S, H], BF16, kind="ExternalOutput")
    outA = nc.dram_tensor("outA", [S, H], BF16, kind="ExternalOutput")

    with tile.TileContext(nc, pool_alloc_mode="stack") as tc:
      for _rep in range(nrep):
          # Pool releases must be LIFO, so the three big input pools
          # (xT/wqk/wv) are created LAST: they release mid-kernel (stack
          # rewinds) and p_wo then reuses their space.
          misc_cm = tc.tile_pool(name="misc", bufs=1); misc = misc_cm.__enter__()
          qkt_cm = tc.tile_pool(name="p_qkt", bufs=1); p_qkt = qkt_cm.__enter__()
          v_cm = tc.tile_pool(name="p_v", bufs=1); p_v = v_cm.__enter__()
          exp_cm = tc.tile_pool(name="p_exp", bufs=3); p_exp = exp_cm.__enter__()
          attn_cm = tc.tile_pool(name="p_attn", bufs=1)
          p_attn = attn_cm.__enter__()
          nrm_cm = tc.tile_pool(name="p_nrm", bufs=2); p_nrm = nrm_cm.__enter__()
          o_cm = tc.tile_pool(name="p_o", bufs=3); p_o = o_cm.__enter__()
          xT_cm = tc.tile_pool(name="p_xT", bufs=1); p_xT = xT_cm.__enter__()
          wqk_cm = tc.tile_pool(name="p_wqk", bufs=1); p_wqk = wqk_cm.__enter__()
          wv_cm = tc.tile_pool(name="p_wv", bufs=1); p_wv = wv_cm.__enter__()
          late = {"p_nrm": p_nrm}
          lgps_cm = tc.tile_pool(name="ps_lg", bufs=2, space="PSUM")
          ps_lg = lgps_cm.__enter__()
          avps_cm = tc.tile_pool(name="ps_av", bufs=2, space="PSUM")
          ps_av = avps_cm.__enter__()
          qkvps_cm = tc.tile_pool(name="ps_qkv", bufs=2, space="PSUM")
          ps_qkv = qkvps_cm.__enter__()

          # ---- input loads: 3 big DMAs (SP) + 1 small (ACT). First
          # proj matmuls need only segment A (wq-b0 + xT-sh0).
          ain = p_xT.tile([128, 20480], BF16, tag="allin")
          nc.sync.dma_start(out=ain[:, 0:1280], in_=allin.ap()[:, 0:1280])
          nc.sync.dma_start(out=ain[:, 1280:3200], in_=allin.ap()[:, 1280:3200])
          nc.sync.dma_start(out=ain[:, 3200:5120], in_=allin.ap()[:, 3200:5120])
          nc.sync.dma_start(out=ain[:, 6144:8192], in_=allin.ap()[:, 6144:8192])
          nc.sync.dma_start(out=ain[:, 5120:6144], in_=allin.ap()[:, 5120:6144])
          nc.sync.dma_start(out=ain[:, 8192:10240], in_=allin.ap()[:, 8192:10240])
          nc.sync.dma_start(out=ain[:, 10240:14336], in_=allin.ap()[:, 10240:14336])
          nc.sync.dma_start(out=ain[:, 14336:20480], in_=allin.ap()[:, 14336:20480])
          sm = misc.tile([128, TC + 8], F32, tag="smalls")
          nc.scalar.dma_start(out=sm, in_=smalls.ap())
          mraw = sm[:, 0:TC]
          maskb = misc.tile([128, TC], F32, tag="maskb")
          nc.vector.tensor_scalar_mul(maskb, mraw, NEG_INF)
          bq_sb = sm[:, TC:TC + 4]
          bk_sb = sm[:, TC + 4:TC + 8]
          bv_row = misc.tile([1, GW], F32, tag="bvrow")
          nc.scalar.dma_start(out=bv_row, in_=bv1[:, :])
          bv_bc = misc.tile([128, GW], F32, tag="bv")
          nc.gpsimd.partition_broadcast(bv_bc, bv_row, channels=128)

          # segment A interleaves [wq0_c (128) | xT-sh0_c (512)] x 8 so the
          # first proj matmuls start after 320KB; then wk-b0 (small, for
          # K0-sh0), xT-sh1, wv (v_chunks are early PE filler), wq/wk b123.
          segA = ain[:, 0:5120].rearrange("p (c g) -> p c g", c=HC)

          def _wq(blk):
              if blk == 0:
                  return segA[:, :, 0:128]
              off = 14336 + (blk - 1) * 1024
              return ain[:, off:off + 1024].rearrange("p (c m) -> p c m", c=HC)

          def _wk(blk):
              if blk == 0:
                  return ain[:, 5120:6144].rearrange("p (c m) -> p c m", c=HC)
              off = 17408 + (blk - 1) * 1024
              return ain[:, off:off + 1024].rearrange("p (c m) -> p c m", c=HC)

          def _xT(sh):
              if sh == 0:
                  return segA[:, :, 128:640]
              return ain[:, 6144:10240].rearrange("p (c s) -> p c s", c=HC)

          wv_sb = ain[:, 10240:14336].rearrange("p (c m) -> p c m", c=HC)
          wo_sb = p_wv.tile([128, 4, S], BF16, tag="wo")
          nc.sync.dma_start(out=wo_sb, in_=wo.ap().rearrange("(c p) n -> p c n", p=128))
          QT_sb = p_qkt.tile([128, 4, S], F32R, tag="QT")
          KT_sb = p_qkt.tile([128, 4, S], F32R, tag="KT")
          V_sb = p_v.tile([128, TC, HPG, HD + 1], EXPTYPE, tag="V")
          nc.gpsimd.memset(V_sb[:, :, :, HD:HD + 1], 1.0)
          late["attnT"] = p_attn.tile([128, 4, S], BF16, tag="attnT", name="attnT")

          def proj_half(dst, blk, sh, wview, b_sb):
              """dst[:, blk, sh-half] (+bias) = block of Wg^T x^T."""
              ps = ps_qkv.tile([128, 512], F32, tag="mm512")
              w = wview(blk)
              xv = _xT(sh)
              for c in range(HC):
                  nc.tensor.matmul(
                      ps, w[:, c, :], xv[:, c, :],
                      start=(c == 0), stop=(c == HC - 1))
              nc.vector.tensor_scalar_add(
                  dst[:, blk, sh * 512:(sh + 1) * 512], ps, b_sb[:, blk:blk + 1])

          def v_chunk(tcn):
              """V_sb[:, tcn, :, 0:64] (+bv) = rows 128*tcn.. of x Wv_g."""
              ps = ps_qkv.tile([128, 512], F32, tag="mm512")
              xv = _xT(tcn // 4)
              ts = (tcn % 4) * 128
              for c in range(HC):
                  nc.tensor.matmul(
                      ps, xv[:, c, ts:ts + 128], wv_sb[:, c, :],
                      start=(c == 0), stop=(c == HC - 1))
              nc.vector.tensor_add(
                  V_sb[:, tcn, :, 0:HD],
                  ps.rearrange("p (h d) -> p h d", h=HPG),
                  bv_bc.rearrange("p (h d) -> p h d", h=HPG))

          def logits_head(h, tcn, expT_h):
              """d=64 logit matmuls (both s-halves) + exp for head h chunk
              tcn."""
              pair, off = h // 2, (h % 2) * 64
              lg = ps_lg.tile([128, 1024], F32, tag="lg")
              for sh in range(SB):
                  nc.tensor.matmul(
                      lg[:, sh * 512:(sh + 1) * 512],
                      KT_sb[off:off + 64, pair, tcn * 128:(tcn + 1) * 128],
                      QT_sb[off:off + 64, pair, sh * 512:(sh + 1) * 512],
                      start=True, stop=True, tile_position=(off, 0))
              nc.scalar.activation(
                  out=expT_h[:, tcn, :], in_=lg, func=Exp,
                  bias=maskb[:, tcn:tcn + 1], scale=0.125)

          def av_head_half(h, expT_h, sh):
              """attnT rows for head h, s-half sh = normalized V_h^T @ expT_h.
              The denominator reciprocal is broadcast across partitions on
              the otherwise-idle Pool engine."""
              off = (h % 2) * 64
              pav = ps_av.tile([HD + 1, 512], F32, tag="av",
                               name=f"pav{h}_{sh}")
              for tcn in range(TC):
                  nc.tensor.matmul(
                      pav, V_sb[:, tcn, h, :],
                      expT_h[:, tcn, sh * 512:(sh + 1) * 512],
                      start=(tcn == 0), stop=(tcn == TC - 1))
              recip = late["p_nrm"].tile([1, 512], F32, tag="recip")
              bcast = late["p_nrm"].tile([HD, 512], F32, tag="bcast")
              att = late["attnT"][off:off + HD, h // 2,
                                  sh * 512:(sh + 1) * 512]
              if h == HPG - 1:
                  # the last head gates the tail: halve the normalize chain
                  # so out-proj st chunks start once their s-columns land
                  for q in range(2):
                      cs = slice(q * 256, (q + 1) * 256)
                      nc.vector.reciprocal(recip[:, cs], pav[HD:HD + 1, cs])
                      nc.gpsimd.partition_broadcast(
                          bcast[:, cs], recip[:, cs], channels=HD)
                      nc.vector.tensor_mul(att[:, cs], pav[0:HD, cs],
                                           bcast[:, cs])
              else:
                  nc.vector.reciprocal(recip, pav[HD:HD + 1, :])
                  nc.gpsimd.partition_broadcast(bcast, recip, channels=HD)
                  nc.vector.tensor_mul(att, pav[0:HD, :], bcast)

          def out_projA(st, nh, blks, dst):
              """partial output (head-pairs 0-1) for chunk (st, nh): runs
              mid-phase through the idle mm512 ring; the host sums the two
              partial outputs, so only head-pairs 2-3 remain for the tail."""
              po = ps_qkv.tile([128, 512], F32, tag="mm512",
                               name=f"poA{st}_{nh}_{blks[0]}")
              for i, blk in enumerate(blks):
                  nc.tensor.matmul(
                      po, late["attnT"][:, blk, st * 128:(st + 1) * 128],
                      wo_sb[:, blk, nh * 512:(nh + 1) * 512],
                      start=(i == 0), stop=(i == len(blks) - 1))
              if nh == 0:
                  late[f"oa{st}"] = p_o.tile([128, 1024], BF16, tag="oa",
                                             name=f"oa{st}")
              oa = late[f"oa{st}"]
              cs = slice(nh * 512, (nh + 1) * 512)
              if (2 * st + nh) % 2 == 0:
                  nc.vector.tensor_copy(oa[:, cs], po)
              else:
                  nc.scalar.copy(oa[:, cs], po)
              if nh == 1:
                  nc.sync.dma_start(out=dst[st * 128:(st + 1) * 128, :], in_=oa)

          def out_proj2(st2):
              """B partial (head-pairs 2-3) for st chunks 2*st2, 2*st2+1:
              two chunks share one SBUF tile and one DMA -- HWDGE and the
              DMA bus are serial and pace the tail."""
              last = st2 == TC // 2 - 1
              o2 = p_o.tile([128, 2, 1024], BF16, tag="o", name=f"o2_{st2}")
              for sl in range(2):
                  st = 2 * st2 + sl
                  pool = late["ps_po"] if st % 3 == 0 else ps_lg
                  po = pool.tile([128, 1024], F32, tag="lg", name=f"po{st}")
                  for nh in range(SB):
                      for blk in (2, 3):
                          nc.tensor.matmul(
                              po[:, nh * 512:(nh + 1) * 512],
                              late["attnT"][:, blk, st * 128:(st + 1) * 128],
                              wo_sb[:, blk, nh * 512:(nh + 1) * 512],
                              start=(blk == 2), stop=(blk == 3))
                  if last:
                      nc.vector.tensor_copy(o2[:, sl, 0:512], po[:, 0:512])
                      nc.scalar.copy(o2[:, sl, 512:1024], po[:, 512:1024])
                  elif sl == 0:
                      nc.vector.tensor_copy(o2[:, 0, :], po)
                  else:
                      nc.scalar.copy(o2[:, 1, :], po)
              if last:
                  for sl in range(2):
                      st = 2 * st2 + sl
                      nc.sync.dma_start(
                          out=out[st * 128:(st + 1) * 128, :], in_=o2[:, sl, :])
              else:
                  nc.sync.dma_start(
                      out=out.ap()[st2 * 256:(st2 + 1) * 256, :]
                            .rearrange("(two p) n -> p two n", two=2),
                      in_=o2)


          # ---------------- emission ----------------
          for sh in range(SB):
              proj_half(QT_sb, 0, sh, _wq, bq_sb)
          for sh in range(SB):
              proj_half(KT_sb, 0, sh, _wk, bk_sb)

          expT = {}
          for h in range(HPG):
              expT[h] = p_exp.tile([128, TC, S], EXPTYPE, tag="expT",
                                   name=f"expT{h}")
              for tcn in range(TC):
                  # interleaved fill work (emission order = scheduler
                  # priority; real ordering comes from the dataflow)
                  if h == 0:
                      if tcn in (0, 2, 4, 6):
                          v_chunk(tcn // 2)
                      elif tcn == 1:
                          proj_half(QT_sb, 1, 0, _wq, bq_sb)
                      elif tcn == 3:
                          proj_half(QT_sb, 1, 1, _wq, bq_sb)
                      elif tcn == 5:
                          proj_half(KT_sb, 1, 0, _wk, bk_sb)
                      elif tcn == 7:
                          proj_half(KT_sb, 1, 1, _wk, bk_sb)
                  elif h == 1 and tcn < 4:
                      v_chunk(4 + tcn)
                  elif h in (2, 4) and tcn in (1, 3, 5, 7):
                      blk = h // 2 + 1
                      wv_, b_sb_ = (_wq, bq_sb) if tcn < 4 else (_wk, bk_sb)
                      dst = QT_sb if tcn < 4 else KT_sb
                      proj_half(dst, blk, (tcn % 4) // 2, wv_, b_sb_)
                  if h == 1:
                      if tcn == 4:
                          av_head_half(0, expT[0], 0)
                      elif tcn == 6:
                          av_head_half(0, expT[0], 1)
                  elif h >= 2:
                      if tcn == 2:
                          av_head_half(h - 1, expT[h - 1], 0)
                      elif tcn == 5:
                          av_head_half(h - 1, expT[h - 1], 1)
                  if h >= 5 and tcn % 2 == 0:
                      u = (h - 5) * 4 + tcn // 2
                      if u < 16:
                          out_projA(u // 2, u % 2, (0, 1), outA)
                  logits_head(h, tcn, expT[h])

          wv_cm.__exit__(None, None, None)
          wqk_cm.__exit__(None, None, None)
          xT_cm.__exit__(None, None, None)

          attnT = late["attnT"]

          for u in range(12, 16):
              out_projA(u // 2, u % 2, (0, 1), outA)
          qkvps_cm.__exit__(None, None, None)
          pops_cm = tc.tile_pool(name="ps_po", bufs=1, space="PSUM")
          late["ps_po"] = pops_cm.__enter__()
          # last head's AVs; the sh1 normalize chain overlaps the st 0-3
          # out-proj matmuls (which only need sh0 rows)
          av_head_half(7, expT[7], 0)
          av_head_half(7, expT[7], 1)
          for st2 in range(TC // 2):
              out_proj2(st2)

          pops_cm.__exit__(None, None, None)
          for cm in (o_cm, nrm_cm, attn_cm, exp_cm, v_cm, qkt_cm,
                     misc_cm, avps_cm, lgps_cm):
              cm.__exit__(None, None, None)

    nc.compile()
    return nc


_NC = {}


def _get_nc(nrep=1):
    if nrep not in _NC:
        _NC[nrep] = _build(nrep)
    return _NC[nrep]


def kernel(x, mask, Wq, bq, Wk, bk, Wv, bv, Wo, bo, _trace=False):
    x = np.asarray(x, dtype=np.float32)
    mask = np.asarray(mask, dtype=np.float32)
    Wq, Wk, Wv, Wo = (np.asarray(w, dtype=np.float32) for w in (Wq, Wk, Wv, Wo))
    bq, bk, bv, bo = (np.asarray(b_, dtype=np.float32) for b_ in (bq, bk, bv, bo))

    nc = _get_nc()

    def _blkmaj(w):
        # [H, GW-slice] -> per-partition [blk 4, c 8, 128]: value (p, blk, c, m)
        # = w[c*128 + p, blk*128 + m]
        r = w.reshape(HC, 128, 4, 128)            # [c, p, blk, m]
        return r.transpose(1, 2, 0, 3)            # [p, blk, c, m]

    def _shmaj(xTb):
        # xT [H, S] -> per-partition [sh 2, c 8, 512]
        r = xTb.reshape(HC, 128, 2, 512)          # [c, p, sh, s]
        return r.transpose(1, 2, 0, 3)            # [p, sh, c, s]

    in_maps = []
    for c in range(NCORES):
        b, g = c // 2, c % 2
        sl = slice(g * GW, (g + 1) * GW)
        bf = ml_dtypes.bfloat16
        wqb = _blkmaj(Wq[:, sl].astype(bf))       # [p, 4, 8, 128]
        wkb = _blkmaj(Wk[:, sl].astype(bf))
        xsh = _shmaj(np.ascontiguousarray(x[b].T).astype(bf))  # [p, 2, 8, 512]
        wvb = Wv[:, sl].astype(bf).reshape(HC, 128, GW).transpose(1, 0, 2)
        segA = np.concatenate([wqb[:, 0], xsh[:, 0]], axis=2)  # [p, c, 640]
        allin = np.concatenate([
            segA.reshape(128, -1),                # wq-b0|xT-sh0 interleaved
            wkb[:, 0].reshape(128, -1),           # wk blk0   1024
            xsh[:, 1].reshape(128, -1),           # xT sh1    4096
            wvb.reshape(128, -1),                 # wv        4096
            wqb[:, 1:4].reshape(128, -1),         # wq blk123 3072
            wkb[:, 1:4].reshape(128, -1),         # wk blk123 3072
        ], axis=1)
        smalls = np.concatenate([
            mask[b, 0, 0, :].reshape(TC, 128).T,
            bq[sl].reshape(4, 128).T,
            bk[sl].reshape(4, 128).T,
        ], axis=1).astype(np.float32)
        in_maps.append({
            "allin": np.ascontiguousarray(allin),
            "wo": np.ascontiguousarray(Wo[sl, :]).astype(bf),
            "smalls": np.ascontiguousarray(smalls),
            "bv1": np.ascontiguousarray(bv[sl]).reshape(1, GW).astype(np.float32),
        })
    # First execution after NEFF load can race engine table initialization
    # (observed: garbage exp output on run 1 only). Warm up, then run.
    run_bass_kernel_spmd(nc, in_maps, core_ids=list(range(NCORES)))
    res = run_bass_kernel_spmd(
        nc, in_maps, core_ids=list(range(NCORES)), trace=_trace)
    kernel.last_results = res
    parts = [np.asarray(res.results[c]["out"]).astype(np.float32)
             + np.asarray(res.results[c]["outA"]).astype(np.float32)
             for c in range(NCORES)]
    return np.stack(
        [parts[2 * b] + parts[2 * b + 1] + bo for b in range(B)]
    ).astype(np.float32)



# revision 110
# speedup vs baseline: 1.0050x; 1.0002x over previous
"""Multi-head attention (B=4, S=1024, H=1024, 16 heads) on 8 TRN2 NeuronCores.

Sharding: core c = (batch b = c//2, head-group g = c%2). Each core computes
attention for its batch over 8 of the 16 heads (a 512-wide column slice of
the QKV projections) plus the matching row-slice of the output projection.
The host sums the partial output projections (2 cores x 2 on-chip partials)
and adds bo.

On-core dataflow (inputs shipped bf16; QK logits in f32r; the attention-
weight matmul in bf16; psum accumulation fp32):
  QT[hd,s] = Wq_g^T x^T (+bq)   KT likewise (+bk)    V[t,hd] = x Wv_g (+bv)
  logitsT[t,s] per head: d=64 contraction, two heads packed in the PE via
                         tile_position row groups; heads processed serially
                         so each head's AV starts a half-pair earlier
  expT = exp(logitsT/8 + mask*NEG_INF)  (bf16; mask as per-partition bias)
  AVT[d,s] & colsum = [V_h | 1]^T @ expT (ones column makes psum row 64 the
                                          softmax denominator)
  attnT = AVT * (1/colsum)   (reciprocal on DVE, partition-broadcast on the
                              otherwise-idle Pool engine)
  out partials: head-pairs 0-1 ship mid-phase as a separate partial output
  (summed on the host) so only head-pairs 2-3 remain after the last AV --
  the tail is paced by the serial DMA bus, so B-chunks pair up per DMA.

Performance notes: all inputs packed into one DRAM tensor (HWDGE descriptor
generation is serial, ~0.63us per DMA; few big DMAs win), first proj blocks
interleave wq/xT per k-chunk so compute starts after 320KB, outputs in bf16.
"""
import sys

sys.path.insert(0, "/opt/trn_rl_repo")

import ml_dtypes
import numpy as np

import concourse.bass as bass
import concourse.mybir as mybir
import concourse.tile as tile
from concourse import bacc
from concourse.bass_utils import run_bass_kernel_spmd

F32 = mybir.dt.float32
F32R = mybir.dt.float32r
BF16 = mybir.dt.bfloat16
EXPTYPE = BF16

B, S, H = 4, 1024, 1024
NH, HD = 16, 64
HPG = 8            # heads per group (per core)
GW = HPG * HD      # 512: group width
NEG_INF = -2.0 ** 32
NCORES = 8
HC = H // 128      # 8 contraction chunks over hidden
TC = S // 128      # 8 chunks over key positions t
SB = S // 512      # 2 halves of the s (query) axis

Exp = mybir.ActivationFunctionType.Exp


def _build(nrep=1):
    nc = bacc.Bacc("TRN2", target_bir_lowering=False, debug=False)

    # all big inputs packed into one tensor (per-partition layout:
    # wq-b0 | xT-sh0 | wk-b0 | xT-sh1 | wq-b123 | wk-b123 | wv), so the
    # whole load is 3 big DMAs -- HWDGE descriptor generation (~0.63us per
    # DMA) is a serial resource.
    allin = nc.dram_tensor("allin", [128, 20480], BF16, kind="ExternalInput")
    wo = nc.dram_tensor("wo", [GW, S], BF16, kind="ExternalInput")
    # mask row | bq | bk (f32): one small DMA
    smalls = nc.dram_tensor("smalls", [128, TC + 8], F32, kind="ExternalInput")
    bv1 = nc.dram_tensor("bv1", [1, GW], F32, kind="ExternalInput")
    out = nc.dram_tensor("out", [S, H], BF16, kind="ExternalOutput")
    outA = nc.dram_tensor("outA", [S, H], BF16, kind="ExternalOutput")

    with tile.TileContext(nc, pool_alloc_mode="stack") as tc:
      for _rep in range(nrep):
          # Pool releases must be LIFO, so the three big input pools
          # (xT/wqk/wv) are created LAST: they release mid-kernel (stack
          # rewinds) and p_wo then reuses their space.
          misc_cm = tc.tile_pool(name="misc", bufs=1); misc = misc_cm.__enter__()
          qkt_cm = tc.tile_pool(name="p_qkt", bufs=1); p_qkt = qkt_cm.__enter__()
          v_cm = tc.tile_pool(name="p_v", bufs=1); p_v = v_cm.__enter__()
          exp_cm = tc.tile_pool(name="p_exp", bufs=3); p_exp = exp_cm.__enter__()
          attn_cm = tc.tile_pool(name="p_attn", bufs=1)
          p_attn = attn_cm.__enter__()
          nrm_cm = tc.tile_pool(name="p_nrm", bufs=2); p_nrm = nrm_cm.__enter__()
          o_cm = tc.tile_pool(name="p_o", bufs=4); p_o = o_cm.__enter__()
          xT_cm = tc.tile_pool(name="p_xT", bufs=1); p_xT = xT_cm.__enter__()
          wqk_cm = tc.tile_pool(name="p_wqk", bufs=1); p_wqk = wqk_cm.__enter__()
          wv_cm = tc.tile_pool(name="p_wv", bufs=1); p_wv = wv_cm.__enter__()
          late = {"p_nrm": p_nrm}
          lgps_cm = tc.tile_pool(name="ps_lg", bufs=2, space="PSUM")
          ps_lg = lgps_cm.__enter__()
          avps_cm = tc.tile_pool(name="ps_av", bufs=2, space="PSUM")
          ps_av = avps_cm.__enter__()
          qkvps_cm = tc.tile_pool(name="ps_qkv", bufs=2, space="PSUM")
          ps_qkv = qkvps_cm.__enter__()

          # ---- input loads: 3 big DMAs (SP) + 1 small (ACT). First
          # proj matmuls need only segment A (wq-b0 + xT-sh0).
          ain = p_xT.tile([128, 20480], BF16, tag="allin")
          nc.sync.dma_start(out=ain[:, 0:1280], in_=allin.ap()[:, 0:1280])
          nc.sync.dma_start(out=ain[:, 1280:3200], in_=allin.ap()[:, 1280:3200])
          nc.sync.dma_start(out=ain[:, 3200:5120], in_=allin.ap()[:, 3200:5120])
          nc.sync.dma_start(out=ain[:, 6144:8192], in_=allin.ap()[:, 6144:8192])
          nc.sync.dma_start(out=ain[:, 5120:6144], in_=allin.ap()[:, 5120:6144])
          nc.sync.dma_start(out=ain[:, 8192:10240], in_=allin.ap()[:, 8192:10240])
          nc.sync.dma_start(out=ain[:, 10240:14336], in_=allin.ap()[:, 10240:14336])
          nc.sync.dma_start(out=ain[:, 14336:20480], in_=allin.ap()[:, 14336:20480])
          sm = misc.tile([128, TC + 8], F32, tag="smalls")
          nc.scalar.dma_start(out=sm, in_=smalls.ap())
          mraw = sm[:, 0:TC]
          maskb = misc.tile([128, TC], F32, tag="maskb")
          nc.vector.tensor_scalar_mul(maskb, mraw, NEG_INF)
          bq_sb = sm[:, TC:TC + 4]
          bk_sb = sm[:, TC + 4:TC + 8]
          bv_row = misc.tile([1, GW], F32, tag="bvrow")
          nc.scalar.dma_start(out=bv_row, in_=bv1[:, :])
          bv_bc = misc.tile([128, GW], F32, tag="bv")
          nc.gpsimd.partition_broadcast(bv_bc, bv_row, channels=128)

          # segment A interleaves [wq0_c (128) | xT-sh0_c (512)] x 8 so the
          # first proj matmuls start after 320KB; then wk-b0 (small, for
          # K0-sh0), xT-sh1, wv (v_chunks are early PE filler), wq/wk b123.
          segA = ain[:, 0:5120].rearrange("p (c g) -> p c g", c=HC)

          def _wq(blk):
              if blk == 0:
                  return segA[:, :, 0:128]
              off = 14336 + (blk - 1) * 1024
              return ain[:, off:off + 1024].rearrange("p (c m) -> p c m", c=HC)

          def _wk(blk):
              if blk == 0:
                  return ain[:, 5120:6144].rearrange("p (c m) -> p c m", c=HC)
              off = 17408 + (blk - 1) * 1024
              return ain[:, off:off + 1024].rearrange("p (c m) -> p c m", c=HC)

          def _xT(sh):
              if sh == 0:
                  return segA[:, :, 128:640]
              return ain[:, 6144:10240].rearrange("p (c s) -> p c s", c=HC)

          wv_sb = ain[:, 10240:14336].rearrange("p (c m) -> p c m", c=HC)
          wo_sb = p_wv.tile([128, 4, S], BF16, tag="wo")
          nc.sync.dma_start(out=wo_sb, in_=wo.ap().rearrange("(c p) n -> p c n", p=128))
          QT_sb = p_qkt.tile([128, 4, S], F32R, tag="QT")
          KT_sb = p_qkt.tile([128, 4, S], F32R, tag="KT")
          V_sb = p_v.tile([128, TC, HPG, HD + 1], EXPTYPE, tag="V")
          nc.gpsimd.memset(V_sb[:, :, :, HD:HD + 1], 1.0)
          late["attnT"] = p_attn.tile([128, 4, S], BF16, tag="attnT", name="attnT")

          def proj_half(dst, blk, sh, wview, b_sb):
              """dst[:, blk, sh-half] (+bias) = block of Wg^T x^T."""
              ps = ps_qkv.tile([128, 512], F32, tag="mm512")
              w = wview(blk)
              xv = _xT(sh)
              for c in range(HC):
                  nc.tensor.matmul(
                      ps, w[:, c, :], xv[:, c, :],
                      start=(c == 0), stop=(c == HC - 1))
              nc.vector.tensor_scalar_add(
                  dst[:, blk, sh * 512:(sh + 1) * 512], ps, b_sb[:, blk:blk + 1])

          def v_chunk(tcn):
              """V_sb[:, tcn, :, 0:64] (+bv) = rows 128*tcn.. of x Wv_g."""
              ps = ps_qkv.tile([128, 512], F32, tag="mm512")
              xv = _xT(tcn // 4)
              ts = (tcn % 4) * 128
              for c in range(HC):
                  nc.tensor.matmul(
                      ps, xv[:, c, ts:ts + 128], wv_sb[:, c, :],
                      start=(c == 0), stop=(c == HC - 1))
              nc.vector.tensor_add(
                  V_sb[:, tcn, :, 0:HD],
                  ps.rearrange("p (h d) -> p h d", h=HPG),
                  bv_bc.rearrange("p (h d) -> p h d", h=HPG))

          def logits_head(h, tcn, expT_h):
              """d=64 logit matmuls (both s-halves) + exp for head h chunk
              tcn."""
              pair, off = h // 2, (h % 2) * 64
              lg = ps_lg.tile([128, 1024], F32, tag="lg")
              for sh in range(SB):
                  nc.tensor.matmul(
                      lg[:, sh * 512:(sh + 1) * 512],
                      KT_sb[off:off + 64, pair, tcn * 128:(tcn + 1) * 128],
                      QT_sb[off:off + 64, pair, sh * 512:(sh + 1) * 512],
                      start=True, stop=True, tile_position=(off, 0))
              nc.scalar.activation(
                  out=expT_h[:, tcn, :], in_=lg, func=Exp,
                  bias=maskb[:, tcn:tcn + 1], scale=0.125)

          def av_head_half(h, expT_h, sh):
              """attnT rows for head h, s-half sh = normalized V_h^T @ expT_h.
              The denominator reciprocal is broadcast across partitions on
              the otherwise-idle Pool engine."""
              off = (h % 2) * 64
              pav = ps_av.tile([HD + 1, 512], F32, tag="av",
                               name=f"pav{h}_{sh}")
              for tcn in range(TC):
                  nc.tensor.matmul(
                      pav, V_sb[:, tcn, h, :],
                      expT_h[:, tcn, sh * 512:(sh + 1) * 512],
                      start=(tcn == 0), stop=(tcn == TC - 1))
              recip = late["p_nrm"].tile([1, 512], F32, tag="recip")
              bcast = late["p_nrm"].tile([HD, 512], F32, tag="bcast")
              att = late["attnT"][off:off + HD, h // 2,
                                  sh * 512:(sh + 1) * 512]
              if h == HPG - 1:
                  # the last head gates the tail: halve the normalize chain
                  # so out-proj st chunks start once their s-columns land
                  for q in range(2):
                      cs = slice(q * 256, (q + 1) * 256)
                      nc.vector.reciprocal(recip[:, cs], pav[HD:HD + 1, cs])
                      nc.gpsimd.partition_broadcast(
                          bcast[:, cs], recip[:, cs], channels=HD)
                      nc.vector.tensor_mul(att[:, cs], pav[0:HD, cs],
                                           bcast[:, cs])
              else:
                  nc.vector.reciprocal(recip, pav[HD:HD + 1, :])
                  nc.gpsimd.partition_broadcast(bcast, recip, channels=HD)
                  nc.vector.tensor_mul(att, pav[0:HD, :], bcast)

          def out_projA(st, nh, blks, dst):
              """partial output (head-pairs 0-1) for chunk (st, nh): runs
              mid-phase through the idle mm512 ring; the host sums the two
              partial outputs, so only head-pairs 2-3 remain for the tail."""
              po = ps_qkv.tile([128, 512], F32, tag="mm512",
                               name=f"poA{st}_{nh}_{blks[0]}")
              for i, blk in enumerate(blks):
                  nc.tensor.matmul(
                      po, late["attnT"][:, blk, st * 128:(st + 1) * 128],
                      wo_sb[:, blk, nh * 512:(nh + 1) * 512],
                      start=(i == 0), stop=(i == len(blks) - 1))
              if nh == 0:
                  late[f"oa{st}"] = p_o.tile([128, 1024], BF16, tag="oa",
                                             name=f"oa{st}")
              oa = late[f"oa{st}"]
              cs = slice(nh * 512, (nh + 1) * 512)
              if (2 * st + nh) % 2 == 0:
                  nc.vector.tensor_copy(oa[:, cs], po)
              else:
                  nc.scalar.copy(oa[:, cs], po)
              if nh == 1:
                  nc.sync.dma_start(out=dst[st * 128:(st + 1) * 128, :], in_=oa)

          def out_proj2(st2):
              """B partial (head-pairs 2-3) for st chunks 2*st2, 2*st2+1:
              two chunks share one SBUF tile and one DMA -- HWDGE and the
              DMA bus are serial and pace the tail."""
              last = st2 == TC // 2 - 1
              o2 = p_o.tile([128, 2, 1024], BF16, tag="o", name=f"o2_{st2}")
              for sl in range(2):
                  st = 2 * st2 + sl
                  pool = late["ps_po"] if st % 3 == 0 else ps_lg
                  po = pool.tile([128, 1024], F32, tag="lg", name=f"po{st}")
                  for nh in range(SB):
                      for blk in (2, 3):
                          nc.tensor.matmul(
                              po[:, nh * 512:(nh + 1) * 512],
                              late["attnT"][:, blk, st * 128:(st + 1) * 128],
                              wo_sb[:, blk, nh * 512:(nh + 1) * 512],
                              start=(blk == 2), stop=(blk == 3))
                  if last:
                      nc.vector.tensor_copy(o2[:, sl, 0:512], po[:, 0:512])
                      nc.scalar.copy(o2[:, sl, 512:1024], po[:, 512:1024])
                  elif sl == 0:
                      nc.vector.tensor_copy(o2[:, 0, :], po)
                  else:
                      nc.scalar.copy(o2[:, 1, :], po)
              if last:
                  for sl in range(2):
                      st = 2 * st2 + sl
                      nc.sync.dma_start(
                          out=out[st * 128:(st + 1) * 128, :], in_=o2[:, sl, :])
              else:
                  nc.sync.dma_start(
                      out=out.ap()[st2 * 256:(st2 + 1) * 256, :]
                            .rearrange("(two p) n -> p two n", two=2),
                      in_=o2)


          # ---------------- emission ----------------
          for sh in range(SB):
              proj_half(QT_sb, 0, sh, _wq, bq_sb)
          for sh in range(SB):
              proj_half(KT_sb, 0, sh, _wk, bk_sb)

          expT = {}
          for h in range(HPG):
              expT[h] = p_exp.tile([128, TC, S], EXPTYPE, tag="expT",
                                   name=f"expT{h}")
              for tcn in range(TC):
                  # interleaved fill work (emission order = scheduler
                  # priority; real ordering comes from the dataflow)
                  if h == 0:
                      if tcn in (0, 2, 4, 6):
                          v_chunk(tcn // 2)
                      elif tcn == 1:
                          proj_half(QT_sb, 1, 0, _wq, bq_sb)
                      elif tcn == 3:
                          proj_half(QT_sb, 1, 1, _wq, bq_sb)
                      elif tcn == 5:
                          proj_half(KT_sb, 1, 0, _wk, bk_sb)
                      elif tcn == 7:
                          proj_half(KT_sb, 1, 1, _wk, bk_sb)
                  elif h == 1 and tcn < 4:
                      v_chunk(4 + tcn)
                  elif h in (2, 4) and tcn in (1, 3, 5, 7):
                      blk = h // 2 + 1
                      wv_, b_sb_ = (_wq, bq_sb) if tcn < 4 else (_wk, bk_sb)
                      dst = QT_sb if tcn < 4 else KT_sb
                      proj_half(dst, blk, (tcn % 4) // 2, wv_, b_sb_)
                  if h == 1:
                      if tcn == 4:
                          av_head_half(0, expT[0], 0)
                      elif tcn == 6:
                          av_head_half(0, expT[0], 1)
                  elif h >= 2:
                      if tcn == 2:
                          av_head_half(h - 1, expT[h - 1], 0)
                      elif tcn == 5:
                          av_head_half(h - 1, expT[h - 1], 1)
                  if h >= 5 and tcn % 2 == 0:
                      u = (h - 5) * 4 + tcn // 2
                      if u < 16:
                          out_projA(u // 2, u % 2, (0, 1), outA)
                  logits_head(h, tcn, expT[h])

          wv_cm.__exit__(None, None, None)
          wqk_cm.__exit__(None, None, None)
          xT_cm.__exit__(None, None, None)

          attnT = late["attnT"]

          for u in range(12, 16):
              out_projA(u // 2, u % 2, (0, 1), outA)
          qkvps_cm.__exit__(None, None, None)
          pops_cm = tc.tile_pool(name="ps_po", bufs=1, space="PSUM")
          late["ps_po"] = pops_cm.__enter__()
          # last head's AVs; the sh1 normalize chain overlaps the st 0-3
          # out-proj matmuls (which only need sh0 rows)
          av_head_half(7, expT[7], 0)
          av_head_half(7, expT[7], 1)
          for st2 in range(TC // 2):
              out_proj2(st2)

          pops_cm.__exit__(None, None, None)
          for cm in (o_cm, nrm_cm, attn_cm, exp_cm, v_cm, qkt_cm,
                     misc_cm, avps_cm, lgps_cm):
              cm.__exit__(None, None, None)

    nc.compile()
    return nc


_NC = {}


def _get_nc(nrep=1):
    if nrep not in _NC:
        _NC[nrep] = _build(nrep)
    return _NC[nrep]


def kernel(x, mask, Wq, bq, Wk, bk, Wv, bv, Wo, bo, _trace=False):
    x = np.asarray(x, dtype=np.float32)
    mask = np.asarray(mask, dtype=np.float32)
    Wq, Wk, Wv, Wo = (np.asarray(w, dtype=np.float32) for w in (Wq, Wk, Wv, Wo))
    bq, bk, bv, bo = (np.asarray(b_, dtype=np.float32) for b_ in (bq, bk, bv, bo))

    nc = _get_nc()

    def _blkmaj(w):
        # [H, GW-slice] -> per-partition [blk 4, c 8, 128]: value (p, blk, c, m)
        # = w[c*128 + p, blk*128 + m]
        r = w.reshape(HC, 128, 4, 128)            # [c, p, blk, m]
        return r.transpose(1, 2, 0, 3)            # [p, blk, c, m]

    def _shmaj(xTb):
        # xT [H, S] -> per-partition [sh 2, c 8, 512]
        r = xTb.reshape(HC, 128, 2, 512)          # [c, p, sh, s]
        return r.transpose(1, 2, 0, 3)            # [p, sh, c, s]

    in_maps = []
    for c in range(NCORES):
        b, g = c // 2, c % 2
        sl = slice(g * GW, (g + 1) * GW)
        bf = ml_dtypes.bfloat16
        wqb = _blkmaj(Wq[:, sl].astype(bf))       # [p, 4, 8, 128]
        wkb = _blkmaj(Wk[:, sl].astype(bf))
        xsh = _shmaj(np.ascontiguousarray(x[b].T).astype(bf))  # [p, 2, 8, 512]
        wvb = Wv[:, sl].astype(bf).reshape(HC, 128, GW).transpose(1, 0, 2)
        segA = np.concatenate([wqb[:, 0], xsh[:, 0]], axis=2)  # [p, c, 640]
        allin = np.concatenate([
            segA.reshape(128, -1),                # wq-b0|xT-sh0 interleaved
            wkb[:, 0].reshape(128, -1),           # wk blk0   1024
            xsh[:, 1].reshape(128, -1),           # xT sh1    4096
            wvb.reshape(128, -1),                 # wv        4096
            wqb[:, 1:4].reshape(128, -1),         # wq blk123 3072
            wkb[:, 1:4].reshape(128, -1),         # wk blk123 3072
        ], axis=1)
        smalls = np.concatenate([
            mask[b, 0, 0, :].reshape(TC, 128).T,
            bq[sl].reshape(4, 128).T,
            bk[sl].reshape(4, 128).T,
        ], axis=1).astype(np.float32)
        in_maps.append({
            "allin": np.ascontiguousarray(allin),
            "wo": np.ascontiguousarray(Wo[sl, :]).astype(bf),
            "smalls": np.ascontiguousarray(smalls),
            "bv1": np.ascontiguousarray(bv[sl]).reshape(1, GW).astype(np.float32),
        })
    # First execution after NEFF load can race engine table initialization
    # (observed: garbage exp output on run 1 only). Warm up, then run.
    run_bass_kernel_spmd(nc, in_maps, core_ids=list(range(NCORES)))
    res = run_bass_kernel_spmd(
        nc, in_maps, core_ids=list(range(NCORES)), trace=_trace)
    kernel.last_results = res
    parts = [np.asarray(res.results[c]["out"]).astype(np.float32)
             + np.asarray(res.results[c]["outA"]).astype(np.float32)
             for c in range(NCORES)]
    return np.stack(
        [parts[2 * b] + parts[2 * b + 1] + bo for b in range(B)]
    ).astype(np.float32)

